# revision 1
# baseline (speedup 1.0000x reference)
"""TRN2 Bass kernel for nn_BetweenClusterFC.

Computes out[n] = sum_f (emb_1 @ W1 + b1)[n,f] * (emb_2 @ W2 + b2)[n,f]
for emb_1/emb_2 [32768, 1024] fp32, W [1024, 512], b [512], out [32768] fp32.

Sharding: data-parallel over the 8 NeuronCores — each core handles 4096 rows;
W1/b1/W2/b2 replicated. No cross-core communication; outputs concatenated on
the host.

Numerics/layout strategy:
  - The embeddings are transposed host-side so each core gets eT [1024, 4096]
    with the contraction dim outermost — matmul lhsT tiles [128 D-chunk,
    128 rows] DMA straight from DRAM (contiguous bursts), eliminating all
    on-device transposes.
  - Each fp32 operand X is split host-side into fp16 hi/lo halves
    (Xh = fp16(X), Xl = fp16(X - Xh); the TRN2 PE handles fp16 subnormals
    exactly, verified on HW). The product is evaluated as three full-rate
    fp16 matmuls accumulated in fp32 PSUM:
        X @ W  ~=  Xh@Wh + Xh@Wl + Xl@Wh     (dropped term is O(2^-22))
    A native fp32 matmul costs 4 PE cycles/row on cayman; the 3-pass fp16
    scheme costs 3 with fp32-grade accuracy (measured ~1.1e-6 max rel err
    vs the fp32 reference, comparable to a pure-fp32 kernel's ~9e-7).
  - Per 128-row tile: the two inputs' 24-matmul accumulation groups are
    interleaved per k-chunk into two PSUM banks (more independent work for
    the PE reorder window); DVE adds the bias, multiplies h1*h2 and reduces
    along the free dim into acc[:, tile]; a final PE transpose of acc
    [128, 32] yields a contiguous [32, 128] store of the 4096 outputs.

Startup: W1 + first tiles load ahead of W2 in consumption order; PE warmup
transposes span the startup-DMA window so real matmuls begin at full clock.
Measured on trn2 (8 cores, SPMD): ~363 us HW exec, max rel err ~1.1e-6.
"""

import sys
import time

import numpy as np

if "/opt/trn_rl_repo" not in sys.path:
    sys.path.insert(0, "/opt/trn_rl_repo")

import concourse.mybir as mybir
import concourse.tile as tile
from concourse import bacc
from concourse.bass_utils import run_bass_kernel_spmd
from concourse.masks import make_identity

F32 = mybir.dt.float32
F16 = mybir.dt.float16

N = 32768
D = 1024
F = 512
P = 128
NCORES = 8
R = N // NCORES  # rows per core
RT = R // P      # 128-row tiles per core
KC = D // P      # contraction chunks

_CACHE = {}


def split_f16(x):
    hi = x.astype(np.float16)
    lo = (x - hi.astype(np.float32)).astype(np.float16)
    return hi, lo


def _build_program(rows=R, compile=True):
    rt_count = rows // P
    nc = bacc.Bacc("TRN2", target_bir_lowering=False, debug=False)

    def din(name, shape, dt=F16):
        return nc.dram_tensor(name, shape, dt, kind="ExternalInput").ap()

    e1h = din("e1h", [D, rows])
    e1l = din("e1l", [D, rows])
    e2h = din("e2h", [D, rows])
    e2l = din("e2l", [D, rows])
    w1h = din("w1h", [D, F])
    w1l = din("w1l", [D, F])
    w2h = din("w2h", [D, F])
    w2l = din("w2l", [D, F])
    b1 = din("b1", [F], F32)
    b2 = din("b2", [F], F32)
    out = nc.dram_tensor("out", [rows], F32, kind="ExternalOutput").ap()

    mult = mybir.AluOpType.mult
    add = mybir.AluOpType.add

    r3 = lambda ap: ap.rearrange("(kc p) r -> p kc r", p=P)
    e1h3, e1l3, e2h3, e2l3 = r3(e1h), r3(e1l), r3(e2h), r3(e2l)

    with tile.TileContext(nc) as tc:
        with (
            tc.tile_pool(name="consts", bufs=1) as consts,
            tc.tile_pool(name="etpool", bufs=3) as etpool,
            tc.tile_pool(name="hpool", bufs=2) as hpool,
            tc.tile_pool(name="fin", bufs=1) as fin_pool,
            tc.tile_pool(name="tp_psum", bufs=1, space="PSUM") as tp_psum,
            tc.tile_pool(name="h_psum", bufs=3, space="PSUM") as h_psum,
        ):
            ident = consts.tile([P, P], F32)
            make_identity(nc, ident)

            w1h_sb = consts.tile([P, KC, F], F16, tag="w1h")
            nc.sync.dma_start(w1h_sb[:], w1h.rearrange("(kc p) f -> p kc f", p=P))
            w1l_sb = consts.tile([P, KC, F], F16, tag="w1l")
            nc.sync.dma_start(w1l_sb[:], w1l.rearrange("(kc p) f -> p kc f", p=P))
            w2h_sb = consts.tile([P, KC, F], F16, tag="w2h")
            w2l_sb = consts.tile([P, KC, F], F16, tag="w2l")

            b1_bc = consts.tile([P, F], F32, tag="b1")
            nc.gpsimd.dma_start(b1_bc[:], b1[None, :].to_broadcast((P, F)))
            b2_bc = consts.tile([P, F], F32, tag="b2")
            nc.gpsimd.dma_start(b2_bc[:], b2[None, :].to_broadcast((P, F)))

            # warm the PE across the whole startup-DMA window so the first
            # real matmuls run at full clock (HAM re-throttles after ~3.4us idle)
            warm_rhs = ident[:, None, :].to_broadcast((P, 4, P))
            warm_ps = h_psum.tile([P, F], F32, tag="h0")
            for _ in range(22):
                nc.tensor.transpose(warm_ps[:], ident[:], warm_rhs)

            acc = fin_pool.tile([P, rt_count], F32, tag="acc")

            for rt in range(rt_count):
                ets, hps = [], []
                for j, (eh3, el3) in enumerate(((e1h3, e1l3), (e2h3, e2l3))):
                    eth = etpool.tile([P, KC, P], F16, tag=f"eth{j}")
                    nc.sync.dma_start(eth[:], eh3[:, :, rt * P:(rt + 1) * P])
                    etl = etpool.tile([P, KC, P], F16, tag=f"etl{j}")
                    nc.sync.dma_start(etl[:], el3[:, :, rt * P:(rt + 1) * P])
                    if rt == 0 and j == 0:
                        nc.sync.dma_start(
                            w2h_sb[:], w2h.rearrange("(kc p) f -> p kc f", p=P))
                        nc.sync.dma_start(
                            w2l_sb[:], w2l.rearrange("(kc p) f -> p kc f", p=P))
                    ets.append((eth, etl))
                    hps.append(h_psum.tile([P, F], F32, tag=f"h{j}", name=f"hp{j}"))

                ws = ((w1h_sb, w1l_sb), (w2h_sb, w2l_sb))
                for kc in range(KC):
                    for j in range(2):
                        (eth, etl), (wh_sb, wl_sb) = ets[j], ws[j]
                        for pi, (lhs, rhs) in enumerate((
                            (eth[:, kc, :], wh_sb[:, kc, :]),
                            (eth[:, kc, :], wl_sb[:, kc, :]),
                            (etl[:, kc, :], wh_sb[:, kc, :]),
                        )):
                            nc.tensor.matmul(
                                hps[j][:], lhsT=lhs, rhs=rhs,
                                start=(kc == 0 and pi == 0),
                                stop=(kc == KC - 1 and pi == 2),
                            )

                hts = []
                for j, b_bc in enumerate((b1_bc, b2_bc)):
                    ht = hpool.tile([P, F], F32, tag=f"ht{j}")
                    nc.vector.tensor_tensor(ht[:], hps[j][:], b_bc[:], add)
                    hts.append(ht)

                prod = hpool.tile([P, F], F32, tag="prod")
                nc.vector.tensor_tensor(prod[:], hts[0][:], hts[1][:], mult)
                nc.vector.tensor_reduce(
                    acc[:, rt:rt + 1], prod[:],
                    axis=mybir.AxisListType.X, op=add,
                )

            # acc [128 rows-in-tile, rt_count tiles] -> out[rt*128 + p]
            ps_fin = tp_psum.tile([rt_count, P], F32, tag="tp")
            nc.tensor.transpose(ps_fin[:], acc[:], ident[:])
            fin = fin_pool.tile([rt_count, P], F32, tag="fin_sb")
            nc.vector.tensor_copy(fin[:], ps_fin[:])
            nc.sync.dma_start(out.rearrange("(rt p) -> rt p", p=P), fin[:])

    if compile:
        nc.compile()
    return nc


def _get_program():
    if "nc" not in _CACHE:
        _CACHE["nc"] = _build_program()
    return _CACHE["nc"]


def make_in_maps(emb_1, emb_2, W1, b1, W2, b2):
    e1t = np.ascontiguousarray(np.asarray(emb_1, dtype=np.float32).T)
    e2t = np.ascontiguousarray(np.asarray(emb_2, dtype=np.float32).T)
    e1h, e1l = split_f16(e1t)
    e2h, e2l = split_f16(e2t)
    w1h, w1l = split_f16(np.ascontiguousarray(np.asarray(W1, dtype=np.float32)))
    w2h, w2l = split_f16(np.ascontiguousarray(np.asarray(W2, dtype=np.float32)))
    b1 = np.ascontiguousarray(np.asarray(b1, dtype=np.float32))
    b2 = np.ascontiguousarray(np.asarray(b2, dtype=np.float32))
    return [
        {
            "e1h": e1h[:, c * R:(c + 1) * R], "e1l": e1l[:, c * R:(c + 1) * R],
            "e2h": e2h[:, c * R:(c + 1) * R], "e2l": e2l[:, c * R:(c + 1) * R],
            "w1h": w1h, "w1l": w1l, "w2h": w2h, "w2l": w2l,
            "b1": b1, "b2": b2,
        }
        for c in range(NCORES)
    ]


def kernel(emb_1, emb_2, W1, b1, W2, b2, **_unused):
    nc = _get_program()
    in_maps = make_in_maps(emb_1, emb_2, W1, b1, W2, b2)
    last_err = None
    for attempt in range(3):
        try:
            res = run_bass_kernel_spmd(nc, in_maps, list(range(NCORES))).results
            return np.concatenate([res[c]["out"] for c in range(NCORES)])
        except Exception as e:  # transient NRT/axon failures observed; retry
            last_err = e
            time.sleep(2.0 * (attempt + 1))
    raise last_err



# revision 5
# speedup vs baseline: 2.7213x; 2.7213x over previous
"""TRN2 Bass kernel for nn_BetweenClusterFC.

Computes out[n] = sum_f (emb_1 @ W1 + b1)[n,f] * (emb_2 @ W2 + b2)[n,f]
for emb_1/emb_2 [32768, 1024] fp32, W [1024, 512], b [512], out [32768] fp32.

Sharding: data-parallel over the 8 NeuronCores — each core handles 4096 rows;
W1/W2 replicated. No cross-core communication; outputs concatenated on the
host.

Strategy (v2 — single-pass fp16):
  - The rel-err budget for this problem is 2e-2; single-pass fp16 matmuls
    land at ~6e-4, so the previous 3-pass hi/lo split (fp32-grade, 3x the PE
    work) is unnecessary. One fp16 matmul per (tile, kc, input) = 512 MMs of
    N=512 per core ~= 262k PE cycles ~= 110us at 2.4 GHz.
  - Embeddings are transposed host-side to [D, rows] fp16 and DMA'd in
    2-row-tile groups [128, 8kc, 256r] so every DMA segment is 512B
    (SDMA line-rate threshold). Weights are split into per-kc-chunk DMAs
    (1KB segments) so the first matmul only waits for ~0.7MB, not 2MB.
  - Per 128-row tile: 16 interleaved matmuls accumulate h1/h2 into two PSUM
    banks; one fused DVE tensor_tensor_reduce computes
    acc[:, rt] = sum_f h1*h2 straight out of PSUM (biases are zero in this
    problem; a general bias variant is compiled only if b1/b2 are nonzero).
  - fp16 warmup matmuls on a zeroed tile bridge the startup-DMA window so
    real matmuls start at full clock (HAM un-throttles after ~3.4us busy).
  - Final PE transpose of acc [128, 32] -> contiguous [32, 128] store.
"""

import sys
import time

import numpy as np

if "/opt/trn_rl_repo" not in sys.path:
    sys.path.insert(0, "/opt/trn_rl_repo")

import concourse.mybir as mybir
import concourse.tile as tile
from concourse import bacc
from concourse.bass_utils import run_bass_kernel_spmd
from concourse.masks import make_identity

F32 = mybir.dt.float32
F16 = mybir.dt.float16

N = 32768
D = 1024
F = 512
P = 128
NCORES = 8
R = N // NCORES   # rows per core
RT = R // P       # 128-row tiles per core
KC = D // P       # contraction chunks
GRP = 2           # row-tiles per e-DMA group (256 rows -> 512B segments)
NG = RT // GRP    # e-DMA groups

_CACHE = {}


def _build_program(with_bias=False, rows=R):
    rt_count = rows // P
    ng = rt_count // GRP
    nc = bacc.Bacc("TRN2", target_bir_lowering=False, debug=False)

    def din(name, shape, dt=F16):
        return nc.dram_tensor(name, shape, dt, kind="ExternalInput").ap()

    e1h = din("e1h", [D, rows])
    e2h = din("e2h", [D, rows])
    w1h = din("w1h", [D, F])
    w2h = din("w2h", [D, F])
    if with_bias:
        b1 = din("b1", [F], F32)
        b2 = din("b2", [F], F32)
    out = nc.dram_tensor("out", [rows], F32, kind="ExternalOutput").ap()

    mult = mybir.AluOpType.mult
    add = mybir.AluOpType.add

    r3 = lambda ap: ap.rearrange("(kc p) r -> p kc r", p=P)
    e1h3, e2h3 = r3(e1h), r3(e2h)
    w1r = w1h.rearrange("(kc p) f -> p kc f", p=P)
    w2r = w2h.rearrange("(kc p) f -> p kc f", p=P)

    with tile.TileContext(nc) as tc:
        with (
            tc.tile_pool(name="consts", bufs=1) as consts,
            tc.tile_pool(name="epool", bufs=1) as epool,
            tc.tile_pool(name="hpool", bufs=2) as hpool,
            tc.tile_pool(name="fin", bufs=1) as fin_pool,
            tc.tile_pool(name="tp_psum", bufs=1, space="PSUM") as tp_psum,
            tc.tile_pool(name="h_psum", bufs=3, space="PSUM") as h_psum,
        ):
            ident = consts.tile([P, P], F32)
            make_identity(nc, ident)

            # startup DMAs in consumption order: first-tile critical first
            w1c = [consts.tile([P, F], F16, tag=f"w1c{k}", name=f"w1c{k}")
                   for k in range(KC)]
            w2c = [consts.tile([P, F], F16, tag=f"w2c{k}", name=f"w2c{k}")
                   for k in range(KC)]
            eg = [
                [epool.tile([P, KC, GRP * P], F16, tag=f"e{j}g{g}",
                            name=f"e{j}g{g}")
                 for g in range(ng)]
                for j in range(2)
            ]

            nc.sync.dma_start(w1c[0][:], w1r[:, 0, :])
            nc.sync.dma_start(w2c[0][:], w2r[:, 0, :])
            nc.sync.dma_start(eg[0][0][:], e1h3[:, :, 0:GRP * P])
            nc.sync.dma_start(eg[1][0][:], e2h3[:, :, 0:GRP * P])
            for k in range(1, KC):
                nc.sync.dma_start(w1c[k][:], w1r[:, k, :])
                nc.sync.dma_start(w2c[k][:], w2r[:, k, :])
            for g in range(1, ng):
                nc.sync.dma_start(eg[0][g][:], e1h3[:, :, g * GRP * P:(g + 1) * GRP * P])
                nc.sync.dma_start(eg[1][g][:], e2h3[:, :, g * GRP * P:(g + 1) * GRP * P])

            if with_bias:
                b1_bc = consts.tile([P, F], F32, tag="b1")
                nc.gpsimd.dma_start(b1_bc[:], b1[None, :].to_broadcast((P, F)))
                b2_bc = consts.tile([P, F], F32, tag="b2")
                nc.gpsimd.dma_start(b2_bc[:], b2[None, :].to_broadcast((P, F)))

            # fp16 warmup matmuls bridge the startup-DMA window (~3.4us) so
            # the first real matmuls run at the un-throttled PE clock
            warm16 = consts.tile([P, F], F16, tag="warm16")
            nc.vector.memset(warm16[:], 0.0)
            warm_ps = tp_psum.tile([P, F], F32, tag="warm")
            for _ in range(9):
                nc.tensor.matmul(
                    warm_ps[:], lhsT=warm16[:, :P], rhs=warm16[:],
                    start=True, stop=True,
                )

            acc = fin_pool.tile([P, rt_count], F32, tag="acc")
            ws = (w1c, w2c)

            for rt in range(rt_count):
                g, ri = divmod(rt, GRP)
                hps = [
                    h_psum.tile([P, F], F32, tag=f"h{j}", name=f"hp{j}_{rt}")
                    for j in range(2)
                ]
                for kc in range(KC):
                    for j in range(2):
                        nc.tensor.matmul(
                            hps[j][:],
                            lhsT=eg[j][g][:, kc, ri * P:(ri + 1) * P],
                            rhs=ws[j][kc][:],
                            start=(kc == 0),
                            stop=(kc == KC - 1),
                        )

                if with_bias:
                    hts = []
                    for j, b_bc in enumerate((b1_bc, b2_bc)):
                        ht = hpool.tile([P, F], F32, tag=f"ht{j}")
                        nc.vector.tensor_tensor(ht[:], hps[j][:], b_bc[:], add)
                        hts.append(ht)
                    in0, in1 = hts[0][:], hts[1][:]
                else:
                    # DVE can read at most one PSUM operand; stage h0 in SBUF
                    # via the scalar engine (close to PSUM, off the DVE path)
                    h0sb = hpool.tile([P, F], F32, tag="h0sb")
                    nc.scalar.activation(
                        h0sb[:], hps[0][:], mybir.ActivationFunctionType.Copy)
                    in0, in1 = h0sb[:], hps[1][:]

                prod = hpool.tile([P, F], F32, tag="prod")
                nc.vector.tensor_tensor(prod[:], in0, in1, mult)
                nc.vector.tensor_reduce(
                    acc[:, rt:rt + 1], prod[:],
                    axis=mybir.AxisListType.X, op=add,
                )

            # acc [128 rows-in-tile, rt_count tiles] -> out[rt*128 + p]
            ps_fin = tp_psum.tile([rt_count, P], F32, tag="tp")
            nc.tensor.transpose(ps_fin[:], acc[:], ident[:])
            fin = fin_pool.tile([rt_count, P], F32, tag="fin_sb")
            nc.vector.tensor_copy(fin[:], ps_fin[:])
            nc.sync.dma_start(out.rearrange("(rt p) -> rt p", p=P), fin[:])

    nc.compile()
    return nc


def _get_program(with_bias=False):
    key = ("bias" if with_bias else "fast")
    if key not in _CACHE:
        _CACHE[key] = _build_program(with_bias=with_bias)
    return _CACHE[key]


def make_in_maps(emb_1, emb_2, W1, b1, W2, b2, with_bias=False):
    e1t = np.ascontiguousarray(
        np.asarray(emb_1, dtype=np.float32).T).astype(np.float16)
    e2t = np.ascontiguousarray(
        np.asarray(emb_2, dtype=np.float32).T).astype(np.float16)
    w1 = np.asarray(W1, dtype=np.float32).astype(np.float16)
    w2 = np.asarray(W2, dtype=np.float32).astype(np.float16)
    maps = []
    for c in range(NCORES):
        m = {
            "e1h": np.ascontiguousarray(e1t[:, c * R:(c + 1) * R]),
            "e2h": np.ascontiguousarray(e2t[:, c * R:(c + 1) * R]),
            "w1h": w1, "w2h": w2,
        }
        if with_bias:
            m["b1"] = np.ascontiguousarray(np.asarray(b1, dtype=np.float32))
            m["b2"] = np.ascontiguousarray(np.asarray(b2, dtype=np.float32))
        maps.append(m)
    return maps


def kernel(emb_1, emb_2, W1, b1, W2, b2, **_unused):
    with_bias = bool(np.any(np.asarray(b1)) or np.any(np.asarray(b2)))
    nc = _get_program(with_bias)
    in_maps = make_in_maps(emb_1, emb_2, W1, b1, W2, b2, with_bias=with_bias)
    last_err = None
    for attempt in range(3):
        try:
            res = run_bass_kernel_spmd(nc, in_maps, list(range(NCORES))).results
            return np.concatenate([res[c]["out"] for c in range(NCORES)])
        except Exception as e:  # transient NRT/axon failures observed; retry
            last_err = e
            time.sleep(2.0 * (attempt + 1))
    raise last_err


# revision 7
# speedup vs baseline: 2.7355x; 1.0052x over previous
"""TRN2 Bass kernel for nn_BetweenClusterFC.

Computes out[n] = sum_f (emb_1 @ W1 + b1)[n,f] * (emb_2 @ W2 + b2)[n,f]
for emb_1/emb_2 [32768, 1024] fp32, W [1024, 512], b [512], out [32768] fp32.

Sharding: data-parallel over the 8 NeuronCores — each core handles 4096 rows;
W1/W2 replicated. No cross-core communication; outputs concatenated on the
host.

Strategy (v3 — single-pass fp16, DMA-descriptor-lean):
  - The rel-err budget for this problem is 2e-2; single-pass fp16 matmuls
    land at ~3.4e-4 (measured on HW), so one fp16 matmul per (tile, kc,
    input) = 512 MMs of N=512 per core runs at the warm-PE stream roofline
    (216 ns/MM measured).
  - Embeddings are relaid out host-side to [group, p, kc, r] so each
    2-row-tile group DMA is 128 descriptors of 4KB contiguous per partition
    (vs 1024x512B from a plain [D, N] transpose — those took 1.6us of sync-
    engine issue time each and stalled the PE stream). Weights are [p, kc, f]
    -> one 128-descriptor DMA per weight matrix.
  - Per 128-row tile: 16 interleaved matmuls accumulate h1/h2 into two PSUM
    banks; the scalar engine stages h1 PSUM->SBUF (DVE may read only one
    PSUM operand); one fused DVE scalar_tensor_tensor computes
    prod = h1*h2 and accum_out = sum_f into acc[:, rt]. Biases are zero in
    this problem; a general bias variant is compiled only if b1/b2 != 0.
  - acc [128 rows-in-tile, 32 tiles] is DMA'd out raw and transposed on the
    host (free), eliminating the PE-transpose + copy + strided-store tail.
  - fp16 warmup matmuls on a zeroed tile bridge the startup-DMA window so
    real matmuls start at the un-throttled PE clock (HAM un-throttles after
    ~3.4us of sustained PE activity).
"""

import sys
import time

import numpy as np

if "/opt/trn_rl_repo" not in sys.path:
    sys.path.insert(0, "/opt/trn_rl_repo")

import concourse.mybir as mybir
import concourse.tile as tile
from concourse import bacc
from concourse.bass_utils import run_bass_kernel_spmd

F32 = mybir.dt.float32
F16 = mybir.dt.float16

N = 32768
D = 1024
F = 512
P = 128
NCORES = 8
R = N // NCORES   # rows per core
RT = R // P       # 128-row tiles per core
KC = D // P       # contraction chunks
GRP = 2           # row-tiles per e-DMA group
NG = RT // GRP    # e-DMA groups
NWARM = 8         # fp16 warmup matmuls bridging the startup DMA window
FUSE_STT = False  # fused DVE mult+reduce crashed HW intermittently; keep off

_CACHE = {}


def _build_program(with_bias=False, rows=R):
    rt_count = rows // P
    ng = rt_count // GRP
    nc = bacc.Bacc("TRN2", target_bir_lowering=False, debug=False)

    def din(name, shape, dt=F16):
        return nc.dram_tensor(name, shape, dt, kind="ExternalInput").ap()

    # host-prearranged layouts: e [group, p, kc, r-in-group], w [p, kc, f]
    e1h = din("e1h", [ng, P, KC, GRP * P])
    e2h = din("e2h", [ng, P, KC, GRP * P])
    w1h = din("w1h", [P, KC, F])
    w2h = din("w2h", [P, KC, F])
    if with_bias:
        b1 = din("b1", [F], F32)
        b2 = din("b2", [F], F32)
    # out[p, rt] = result for row rt*128+p; transposed host-side
    out = nc.dram_tensor("out", [P, rt_count], F32, kind="ExternalOutput").ap()

    mult = mybir.AluOpType.mult
    add = mybir.AluOpType.add

    with tile.TileContext(nc) as tc:
        with (
            tc.tile_pool(name="consts", bufs=1) as consts,
            tc.tile_pool(name="epool", bufs=1) as epool,
            tc.tile_pool(name="hpool", bufs=2) as hpool,
            tc.tile_pool(name="fin", bufs=1) as fin_pool,
            tc.tile_pool(name="w_psum", bufs=1, space="PSUM") as w_psum,
            tc.tile_pool(name="h_psum", bufs=3, space="PSUM") as h_psum,
        ):
            w1sb = consts.tile([P, KC, F], F16, tag="w1sb")
            w2sb = consts.tile([P, KC, F], F16, tag="w2sb")
            eg = [
                [epool.tile([P, KC, GRP * P], F16, tag=f"e{j}g{g}",
                            name=f"e{j}g{g}")
                 for g in range(ng)]
                for j in range(2)
            ]

            # issue order = consumption order; each is 128 descriptors
            nc.sync.dma_start(w1sb[:], w1h)
            nc.sync.dma_start(w2sb[:], w2h)
            nc.sync.dma_start(eg[0][0][:], e1h[0])
            nc.sync.dma_start(eg[1][0][:], e2h[0])
            for g in range(1, ng):
                nc.sync.dma_start(eg[0][g][:], e1h[g])
                nc.sync.dma_start(eg[1][g][:], e2h[g])

            if with_bias:
                b1_bc = consts.tile([P, F], F32, tag="b1")
                nc.gpsimd.dma_start(b1_bc[:], b1[None, :].to_broadcast((P, F)))
                b2_bc = consts.tile([P, F], F32, tag="b2")
                nc.gpsimd.dma_start(b2_bc[:], b2[None, :].to_broadcast((P, F)))

            # fp16 warmup matmuls bridge the startup-DMA window (~3us) so
            # the first real matmuls run at the un-throttled PE clock
            warm16 = consts.tile([P, F], F16, tag="warm16")
            nc.vector.memset(warm16[:], 0.0)
            warm_ps = w_psum.tile([P, F], F32, tag="warm")
            for _ in range(NWARM):
                nc.tensor.matmul(
                    warm_ps[:], lhsT=warm16[:, :P], rhs=warm16[:],
                    start=True, stop=True,
                )

            acc = fin_pool.tile([P, rt_count], F32, tag="acc")
            ws = (w1sb, w2sb)

            for rt in range(rt_count):
                g, ri = divmod(rt, GRP)
                hps = [
                    h_psum.tile([P, F], F32, tag=f"h{j}", name=f"hp{j}_{rt}")
                    for j in range(2)
                ]
                for kc in range(KC):
                    for j in range(2):
                        nc.tensor.matmul(
                            hps[j][:],
                            lhsT=eg[j][g][:, kc, ri * P:(ri + 1) * P],
                            rhs=ws[j][:, kc, :],
                            start=(kc == 0),
                            stop=(kc == KC - 1),
                        )

                if with_bias:
                    hts = []
                    for j, b_bc in enumerate((b1_bc, b2_bc)):
                        ht = hpool.tile([P, F], F32, tag=f"ht{j}")
                        nc.vector.tensor_tensor(ht[:], hps[j][:], b_bc[:], add)
                        hts.append(ht)
                    in0, in1 = hts[0][:], hts[1][:]
                else:
                    # DVE can read at most one PSUM operand; stage h0 in SBUF
                    # via the scalar engine (close to PSUM, off the DVE path)
                    h0sb = hpool.tile([P, F], F32, tag="h0sb")
                    nc.scalar.activation(
                        h0sb[:], hps[0][:], mybir.ActivationFunctionType.Copy)
                    in0, in1 = h0sb[:], hps[1][:]

                prod = hpool.tile([P, F], F32, tag="prod")
                if FUSE_STT:
                    nc.vector.scalar_tensor_tensor(
                        prod[:], in0, 1.0, in1, op0=mult, op1=mult,
                        accum_out=acc[:, rt:rt + 1],
                    )
                else:
                    nc.vector.tensor_tensor(prod[:], in0, in1, mult)
                    nc.vector.tensor_reduce(
                        acc[:, rt:rt + 1], prod[:],
                        axis=mybir.AxisListType.X, op=add,
                    )

            nc.sync.dma_start(out, acc[:])

    nc.compile()
    return nc


def _get_program(with_bias=False):
    key = ("bias" if with_bias else "fast")
    if key not in _CACHE:
        _CACHE[key] = _build_program(with_bias=with_bias)
    return _CACHE[key]


def _prep_e(emb):
    # [N, D] fp32 -> per-core [ng, p, kc, r] fp16, 4KB contiguous/partition
    et = np.ascontiguousarray(
        np.asarray(emb, dtype=np.float32).T).astype(np.float16)
    # et [D, N]: [kc*128+p, c*R + g*GRP*P + r]
    v = et.reshape(KC, P, NCORES, NG, GRP * P)
    return np.ascontiguousarray(v.transpose(2, 3, 1, 0, 4))  # [c, g, p, kc, r]


def _prep_w(W):
    w = np.asarray(W, dtype=np.float32).astype(np.float16)
    return np.ascontiguousarray(w.reshape(KC, P, F).transpose(1, 0, 2))


def make_in_maps(emb_1, emb_2, W1, b1, W2, b2, with_bias=False):
    e1 = _prep_e(emb_1)
    e2 = _prep_e(emb_2)
    w1 = _prep_w(W1)
    w2 = _prep_w(W2)
    maps = []
    for c in range(NCORES):
        m = {"e1h": e1[c], "e2h": e2[c], "w1h": w1, "w2h": w2}
        if with_bias:
            m["b1"] = np.ascontiguousarray(np.asarray(b1, dtype=np.float32))
            m["b2"] = np.ascontiguousarray(np.asarray(b2, dtype=np.float32))
        maps.append(m)
    return maps


def kernel(emb_1, emb_2, W1, b1, W2, b2, **_unused):
    with_bias = bool(np.any(np.asarray(b1)) or np.any(np.asarray(b2)))
    nc = _get_program(with_bias)
    in_maps = make_in_maps(emb_1, emb_2, W1, b1, W2, b2, with_bias=with_bias)
    last_err = None
    for attempt in range(3):
        try:
            res = run_bass_kernel_spmd(nc, in_maps, list(range(NCORES))).results
            # out[p, rt] -> rows rt*128+p
            return np.concatenate(
                [res[c]["out"].T.reshape(R) for c in range(NCORES)])
        except Exception as e:  # transient NRT/axon failures observed; retry
            last_err = e
            time.sleep(2.0 * (attempt + 1))
    raise last_err
